# revision 13
# baseline (speedup 1.0000x reference)
"""Trainium2 Bass kernel for nn_MultiHeadAttention (B=4, S=2048, D=512, H=8).

Sharding: tensor-parallel over heads — core c owns head c (Dh=64).
Each core computes q/k/v projections for its head slice (full x replicated,
host-pre-transposed to x^T in bf16), attention for its head over all 4
batches, and the partial out-projection O_c @ Wo[c]. Softmax denominators
are accumulated by zero-padded 64-wide ones-weight matmuls that co-stream
with the opposite batch's AV matmul; the host divides each partial by its
denominator and sums the 8 partials (the TP all-reduce done at gather
time), adding the biases that commute with that reduction (bo, bv@Wo).

Engine plan: ACT does exclusively exp (128 x [128,1024] instrs, ~1.15us
each — the pacing engine, ~147us floor). The PE issues one 512-col matmul
per ~216ns slot and co-streams two instructions whose footprints are
disjoint 64-row or 64-col groups. Steady-state PE load per key tile is 3
pair-slots (S^T batch pair, AV0||ones1, AV1||ones0) ~= 650ns < 1146ns exp
cadence. All other PE work is shaped into pair-slots too (q/k/v prep batch
pairs on column groups, V-transpose batch pairs on row groups, out-proj
batch pairs on row groups against Wo duplicated on both partition halves)
and drip-fed between key tiles from unit queues under a per-tile slot
budget with forced due-dates, so the PE never delays the next exp. x^T
loads are block-major; dummy warm-up matmuls ramp the PE clock while block
0 streams in.
"""
import numpy as np

import concourse.bass as bass
import concourse.mybir as mybir
import concourse.tile as tile
from concourse import bacc
from concourse.bass_utils import run_bass_kernel_spmd

B, S, D = 4, 2048, 512
H, DH = 8, 64
NCORES = 8
F32 = mybir.dt.float32
BF16 = mybir.dt.bfloat16
AF = mybir.ActivationFunctionType

NKT = S // 128          # 16 key tiles per batch
NQB = S // 512          # 4 query blocks per batch
NCH = D // 128          # 4 d_model chunks

_NC_CACHE = {}


def build_kernel():
    nc = bacc.Bacc("TRN2", target_bir_lowering=False, debug=False)

    xT = nc.dram_tensor("xT", [B, D, S], BF16, kind="ExternalInput")
    wq = nc.dram_tensor("wq", [D, DH], BF16, kind="ExternalInput")
    wk = nc.dram_tensor("wk", [D, DH], BF16, kind="ExternalInput")
    wv = nc.dram_tensor("wv", [D, DH], BF16, kind="ExternalInput")
    wo_dup = nc.dram_tensor("wo_dup", [128, D], BF16, kind="ExternalInput")
    bq = nc.dram_tensor("bq", [128, 1], F32, kind="ExternalInput")
    bk = nc.dram_tensor("bk", [128, 1], F32, kind="ExternalInput")
    idin = nc.dram_tensor("idin", [128, 128], BF16, kind="ExternalInput")
    onesw = nc.dram_tensor("onesw", [128, DH], BF16, kind="ExternalInput")
    out = nc.dram_tensor("out", [B * S, D], BF16, kind="ExternalOutput")
    den = nc.dram_tensor("den", [B, S], F32, kind="ExternalOutput")

    with tile.TileContext(nc) as tc:
        with (
            tc.tile_pool(name="consts", bufs=1) as consts,
            tc.tile_pool(name="xtp", bufs=16) as xtp,
            tc.tile_pool(name="qkp", bufs=2) as qkp,
            tc.tile_pool(name="vtp", bufs=2) as vtp,
            tc.tile_pool(name="vp", bufs=4) as vp,
            tc.tile_pool(name="ptp", bufs=3) as ptp,
            tc.tile_pool(name="otp", bufs=2) as otp,
            tc.tile_pool(name="dnp", bufs=2) as dnp,
            tc.tile_pool(name="outp", bufs=6) as outp,
            tc.tile_pool(name="psA", bufs=2, space="PSUM") as psA,   # pst [128,1024] x2 = 4 banks
            tc.tile_pool(name="psO", bufs=1, space="PSUM") as psO,   # po  [128,512]     1 bank
            tc.tile_pool(name="psD", bufs=1, space="PSUM") as psDp,  # den [128,512]     1 bank
            tc.tile_pool(name="psM", bufs=2, space="PSUM") as psM,   # misc [128,512] x2 2 banks
        ):
            wq_sb = consts.tile([128, NCH, DH], BF16)
            wk_sb = consts.tile([128, NCH, DH], BF16)
            wv_sb = consts.tile([128, NCH, DH], BF16)
            wo_sb = consts.tile([128, D], BF16)
            ones_sb = consts.tile([128, DH], BF16)
            bq_sb = consts.tile([128, 1], F32)
            bk_sb = consts.tile([128, 1], F32)
            ident = consts.tile([128, 128], BF16)
            nc.sync.dma_start(out=wq_sb[:], in_=wq.rearrange("(c p) m -> p c m", p=128))
            nc.sync.dma_start(out=wk_sb[:], in_=wk.rearrange("(c p) m -> p c m", p=128))
            nc.sync.dma_start(out=wv_sb[:], in_=wv.rearrange("(c p) m -> p c m", p=128))
            nc.sync.dma_start(out=ones_sb[:], in_=onesw[:])
            nc.gpsimd.dma_start(out=ident[:], in_=idin[:])
            nc.gpsimd.dma_start(out=wo_sb[:], in_=wo_dup[:])
            nc.gpsimd.dma_start(out=bq_sb[:], in_=bq[:])
            nc.gpsimd.dma_start(out=bk_sb[:], in_=bk[:])

            state = {}

            def alloc_pair(pr):
                st = {"xt": {0: [], 1: []}, "v": {}}
                st["qt"] = qkp.tile([128, S], BF16, tag="qt", name=f"qt_{pr}")
                st["kt"] = qkp.tile([128, S], BF16, tag="kt", name=f"kt_{pr}")
                st["vt"] = vtp.tile([128, S], BF16, tag="vt", name=f"vt_{pr}")
                st["dn"] = dnp.tile([65, 2, S], F32, tag="dn", name=f"dn_{pr}")
                st["ot"] = otp.tile([128, S], BF16, tag="ot", name=f"ot_{pr}")
                for half in range(2):
                    st["v"][half] = vp.tile([128, NKT, DH], BF16, tag="v", name=f"v_{pr}_{half}")
                for half in range(2):
                    for ci in range(NCH):
                        st["xt"][half].append(
                            xtp.tile([128, S], BF16, tag="xt", name=f"xt_{pr}_{half}_{ci}")
                        )
                state[pr] = st

            def emit_xt_loads(pr):
                # blk-major so block 0 of BOTH halves lands first
                st = state[pr]
                for blk in range(NQB):
                    for half in range(2):
                        b = pr * 2 + half
                        for ci in range(NCH):
                            eng = nc.sync if (ci % 2 == 0) else nc.gpsimd
                            eng.dma_start(
                                out=st["xt"][half][ci][:, bass.ts(blk, 512)],
                                in_=xT[b, bass.ts(ci, 128), bass.ts(blk, 512)],
                            )

            def emit_prep(pr, blk, which, ci):
                """One unit = one d_model chunk for both halves (one PE slot)."""
                st = state[pr]
                sl = bass.ts(blk, 512)
                w_sb = {"q": wq_sb, "k": wk_sb, "v": wv_sb}[which]
                key = f"p{which}_{pr}_{blk}"
                if ci == 0:
                    st[key] = psM.tile([128, 512], F32, tag="m", name=key)
                pp = st[key]
                for half in range(2):
                    nc.tensor.matmul(
                        pp[half * DH:(half + 1) * DH, :],
                        w_sb[:, ci, :], st["xt"][half][ci][:, sl],
                        start=(ci == 0), stop=(ci == NCH - 1),
                        tile_position=(0, half * DH),
                        skip_group_check=True,
                    )
                if ci == NCH - 1:
                    if which == "q":
                        nc.vector.tensor_scalar_add(st["qt"][:, sl], pp[:], bq_sb[:])
                    elif which == "k":
                        nc.vector.tensor_scalar_add(st["kt"][:, sl], pp[:], bk_sb[:])
                    else:
                        nc.vector.tensor_copy(st["vt"][:, sl], pp[:])

            def emit_vtr(pr, g):
                """Transpose V^T -> V for key tiles g*4..g*4+3, both halves."""
                st = state[pr]
                pv0 = psM.tile([128, 256], BF16, tag="m", name=f"pvtr0_{pr}_{g}")
                pv1 = psM.tile([128, 256], BF16, tag="m", name=f"pvtr1_{pr}_{g}")
                for j in range(4):
                    nc.tensor.transpose(
                        pv0[:, bass.ts(j, 64)],
                        st["vt"][0:DH, bass.ts(g * 4 + j, 128)],
                        ident[0:DH, 0:DH],
                        tile_position=(0, 0),
                    )
                    nc.tensor.transpose(
                        pv1[:, bass.ts(j, 64)],
                        st["vt"][DH:128, bass.ts(g * 4 + j, 128)],
                        ident[DH:128, DH:128],
                        tile_position=(64, 0),
                    )
                nc.vector.tensor_copy(
                    st["v"][0][:, bass.ds(g * 4, 4), :],
                    pv0[:].rearrange("p (k m) -> p k m", m=64),
                )
                nc.vector.tensor_copy(
                    st["v"][1][:, bass.ds(g * 4, 4), :],
                    pv1[:].rearrange("p (k m) -> p k m", m=64),
                )

            def emit_op(pr, tt):
                """Out-projection for one token tile, both batches (row pair)."""
                st = state[pr]
                otc = st["ot"]
                pop0 = psM.tile([128, 512], F32, tag="m", name=f"pop0_{pr}_{tt}")
                pop1 = psM.tile([128, 512], F32, tag="m", name=f"pop1_{pr}_{tt}")
                nc.tensor.matmul(
                    pop0[:], otc[0:DH, bass.ts(tt, 128)], wo_sb[0:DH, :],
                    start=True, stop=True, tile_position=(0, 0),
                )
                nc.tensor.matmul(
                    pop1[:], otc[DH:128, bass.ts(tt, 128)], wo_sb[DH:128, :],
                    start=True, stop=True, tile_position=(64, 0),
                )
                so0 = outp.tile([128, 512], BF16, tag="so", name=f"so0_{pr}_{tt}")
                so1 = outp.tile([128, 512], BF16, tag="so", name=f"so1_{pr}_{tt}")
                nc.vector.tensor_copy(so0[:], pop0[:])
                nc.vector.tensor_copy(so1[:], pop1[:])
                b0 = pr * 2
                b1 = pr * 2 + 1
                nc.sync.dma_start(out=out[bass.ds(b0 * S + tt * 128, 128), :], in_=so0[:])
                nc.gpsimd.dma_start(out=out[bass.ds(b1 * S + tt * 128, 128), :], in_=so1[:])

            # ---- filler unit queues: (weight_in_slots, due_kt, fn) ----
            fq = []
            oq = []

            def pump(q, budget, carry, kt_i):
                budget += carry
                while q and (q[0][1] <= kt_i or q[0][0] <= budget):
                    w, due, fn = q.pop(0)
                    fn()
                    budget -= w
                return min(max(budget, 0.0), 4.0)

            pend = []   # AV+den quads run two key tiles behind their exp so
                        # the in-order PE never blocks the next S^T; carried
                        # across query blocks (evacs ride the last quad)

            def pop_pend():
                e = pend.pop(0)
                e[0]()
                if e[1] is not None:
                    e[1]()

            def flush_pend():
                while pend:
                    pop_pend()

            def emit_attn(pr, qq, f_budget=0.0, o_budget=0.0):
                st = state[pr]
                fcarry = ocarry = 0.0
                with nc.named_scope(f"attn_{pr}_{qq}"):
                    sl_q = bass.ts(qq, 512)
                    po = psO.tile([128, 512], F32, tag="po", name=f"po_{pr}_{qq}")
                    psd = psDp.tile([128, 512], F32, tag="pd", name=f"psd_{pr}_{qq}")

                    def emit_av(kt_i, ptt):
                        first = kt_i == 0
                        last = kt_i == NKT - 1
                        nc.tensor.matmul(
                            po[0:DH, :], st["v"][0][:, kt_i, :], ptt[:, 0:512],
                            start=first, stop=last, tile_position=(0, 0),
                            skip_group_check=True,
                        )
                        nc.tensor.matmul(
                            psd[DH:128, :], ones_sb[:], ptt[:, 512:1024],
                            start=first, stop=last, tile_position=(0, 64),
                            skip_group_check=True,
                        )
                        nc.tensor.matmul(
                            po[DH:128, :], st["v"][1][:, kt_i, :], ptt[:, 512:1024],
                            start=first, stop=last, tile_position=(0, 64),
                            skip_group_check=True,
                        )
                        nc.tensor.matmul(
                            psd[0:DH, :], ones_sb[:], ptt[:, 0:512],
                            start=first, stop=last, tile_position=(0, 0),
                            skip_group_check=True,
                        )

                    def emit_evac():
                        nc.vector.tensor_copy(st["ot"][:, sl_q], po[:])
                        nc.vector.tensor_copy(st["dn"][0:1, 0, sl_q], psd[0:1, :])
                        nc.vector.tensor_copy(st["dn"][64:65, 1, sl_q], psd[64:65, :])

                    import functools as _ft
                    for kt_i in range(NKT):
                        kt_sl = bass.ts(kt_i, 128)
                        pst = psA.tile([128, 1024], F32, tag="pst", name=f"pst_{pr}_{qq}_{kt_i}")
                        for hb in range(2):
                            nc.tensor.matmul(
                                pst[:, bass.ts(hb, 512)],
                                st["kt"][hb * DH:(hb + 1) * DH, kt_sl],
                                st["qt"][hb * DH:(hb + 1) * DH, sl_q],
                                start=True, stop=True,
                                tile_position=(hb * DH, 0),
                            )
                        ptt = ptp.tile([128, 1024], BF16, tag="pt", name=f"ptt_{pr}_{qq}_{kt_i}")
                        nc.scalar.activation(ptt[:], pst[:], AF.Exp, scale=0.125)
                        pend.append((
                            _ft.partial(emit_av, kt_i, ptt),
                            emit_evac if kt_i == NKT - 1 else None,
                        ))
                        if len(pend) > 2:
                            pop_pend()
                        fcarry = pump(fq, f_budget, fcarry, kt_i)
                        ocarry = pump(oq, o_budget, ocarry, kt_i)

            def emit_den_out(pr):
                st = state[pr]
                nc.gpsimd.dma_start(out=den[bass.ds(pr * 2, 1), :], in_=st["dn"][0:1, 0, :])
                nc.gpsimd.dma_start(out=den[bass.ds(pr * 2 + 1, 1), :], in_=st["dn"][64:65, 1, :])

            # ---------------- emission schedule ----------------
            import functools
            P = functools.partial
            alloc_pair(0)
            alloc_pair(1)
            emit_xt_loads(0)
            emit_xt_loads(1)

            # PE warmup: ramp the clock while x^T streams in (results unused)
            dummy = psM.tile([128, 256], F32, tag="m", name="dummy")
            for _ in range(6):
                nc.tensor.matmul(
                    dummy[:], ident[:], wq_sb[:].rearrange("p c m -> p (c m)"),
                    start=True, stop=True,
                )

            # head: pair-0 block-0 prep + first V rung
            for w in ("q", "k", "v"):
                for ci in range(NCH):
                    emit_prep(0, 0, w, ci)
            emit_vtr(0, 0)

            def prep_units(pr, blk, which, due=99):
                return [(1.0, due, P(emit_prep, pr, blk, which, ci)) for ci in range(NCH)]

            def drain(q):
                while q:
                    q.pop(0)[2]()

            # attn(0,0) fillers: k/v/vtr ladder blks 1-3 + q(0,1) for attn(0,1)
            fq += prep_units(0, 1, "k", 0) + prep_units(0, 1, "v", 1)
            fq += [(4.0, 2, P(emit_vtr, 0, 1))]
            fq += prep_units(0, 2, "k", 4) + prep_units(0, 2, "v", 5)
            fq += [(4.0, 6, P(emit_vtr, 0, 2))]
            fq += prep_units(0, 3, "k", 8) + prep_units(0, 3, "v", 9)
            fq += [(4.0, 10, P(emit_vtr, 0, 3))]
            fq += prep_units(0, 1, "q")
            emit_attn(0, 0, f_budget=2.2)
            drain(fq)

            fq += prep_units(0, 2, "q")
            fq += prep_units(1, 0, "k") + prep_units(1, 0, "v") + prep_units(1, 0, "q")
            fq += [(4.0, 99, P(emit_vtr, 1, 0))]
            fq += prep_units(1, 1, "k") + prep_units(1, 1, "v")
            emit_attn(0, 1, f_budget=1.7)
            drain(fq)

            fq += prep_units(0, 3, "q")
            fq += prep_units(1, 1, "q")
            fq += [(4.0, 99, P(emit_vtr, 1, 1))]
            fq += prep_units(1, 2, "k") + prep_units(1, 2, "v") + prep_units(1, 2, "q")
            fq += [(4.0, 99, P(emit_vtr, 1, 2))]
            emit_attn(0, 2, f_budget=1.7)
            drain(fq)

            fq += prep_units(1, 3, "k") + prep_units(1, 3, "v") + prep_units(1, 3, "q")
            fq += [(4.0, 99, P(emit_vtr, 1, 3))]
            emit_attn(0, 3, f_budget=1.7)
            drain(fq)

            # out-projections: pr0 + pr1(qq<3) during attn(1,*), pr1 qq3 in tail
            for tt in range(NKT):
                oq.append((1.0, 99, P(emit_op, 0, tt)))
            emit_attn(1, 0, o_budget=1.2)
            emit_den_out(0)
            for tt in range(4):
                oq.append((1.0, 99, P(emit_op, 1, tt)))
            emit_attn(1, 1, o_budget=0.8)
            for tt in range(4, 8):
                oq.append((1.0, 99, P(emit_op, 1, tt)))
            emit_attn(1, 2, o_budget=0.8)
            for tt in range(8, 12):
                oq.append((1.0, 99, P(emit_op, 1, tt)))
            emit_attn(1, 3, o_budget=0.8)
            flush_pend()
            drain(oq)
            for tt in range(12, 16):
                emit_op(1, tt)
            emit_den_out(1)

    nc.compile()
    return nc


def kernel(x, Wq, bq, Wk, bk, Wv, bv, Wo, bo):
    import ml_dtypes
    x = np.asarray(x, dtype=np.float32)
    xT = np.ascontiguousarray(np.transpose(x, (0, 2, 1))).astype(ml_dtypes.bfloat16)
    Wq = np.asarray(Wq, dtype=np.float32)
    Wk = np.asarray(Wk, dtype=np.float32)
    Wv = np.asarray(Wv, dtype=np.float32)
    Wo = np.asarray(Wo, dtype=np.float32)
    bq = np.asarray(bq, dtype=np.float32)
    bk = np.asarray(bk, dtype=np.float32)
    bv = np.asarray(bv, dtype=np.float32)
    bo = np.asarray(bo, dtype=np.float32)

    if "nc" not in _NC_CACHE:
        _NC_CACHE["nc"] = build_kernel()
    nc = _NC_CACHE["nc"]

    eye = np.eye(128).astype(ml_dtypes.bfloat16)
    onesw = np.zeros((128, DH), dtype=ml_dtypes.bfloat16)
    onesw[:, 0] = 1.0
    in_maps = []
    for c in range(NCORES):
        hs = slice(c * DH, (c + 1) * DH)
        in_maps.append({
            "xT": xT,
            "wq": np.ascontiguousarray(Wq[:, hs]).astype(ml_dtypes.bfloat16),
            "wk": np.ascontiguousarray(Wk[:, hs]).astype(ml_dtypes.bfloat16),
            "wv": np.ascontiguousarray(Wv[:, hs]).astype(ml_dtypes.bfloat16),
            "wo_dup": np.ascontiguousarray(
                np.concatenate([Wo[hs, :], Wo[hs, :]], axis=0)
            ).astype(ml_dtypes.bfloat16),
            "bq": np.ascontiguousarray(np.concatenate([bq[hs], bq[hs]]).reshape(128, 1)),
            "bk": np.ascontiguousarray(np.concatenate([bk[hs], bk[hs]]).reshape(128, 1)),
            "idin": eye,
            "onesw": onesw,
        })

    res = run_bass_kernel_spmd(nc, in_maps, list(range(NCORES)))

    acc = np.zeros((B * S, D), dtype=np.float32)
    for c in range(NCORES):
        o = np.asarray(res.results[c]["out"], dtype=np.float32)
        d = np.asarray(res.results[c]["den"], dtype=np.float32).reshape(B * S, 1)
        acc += o / d
    # biases that commute with the head-reduction, applied at gather time
    acc += bo[None, :] + (bv @ Wo)[None, :]
    return acc.reshape(B, S, D)
